# revision 1
# baseline (speedup 1.0000x reference)
"""DIST loss (hard CE + inter/intra Pearson distillation) on 8 Trainium2 cores.

Strategy: data-parallel over the batch dim (4096 rows -> 512 rows/core).
Each core streams its [512, 32000] f32 shard of z_s/z_t once from HBM,
computes exp() on the ScalarE (caching bf16 exponentials in SBUF), then
produces:
  - per-row stats  [512, 5]: Zs, Zt, U11=sum(es^2), U22=sum(et^2), U12=sum(es*et)
    (U11 comes free from the ScalarE Square activation's accumulator;
     U22/U12 via VectorE halve-add + reduce)
  - per-column weighted partial sums (one slab per 128-row block):
    S1=sum(es/Zs), S2=sum(et/Zt), S11=sum(es^2/Zs^2), S22=sum(et^2/Zt^2),
    S12=sum(es*et/(Zs*Zt)) -- TensorE matmuls with zero-padded per-stat
    weight columns as the stationary operand; the three 512-col sub-matmuls
    of a chunk land at PSUM base partitions 0/32/64 of a single bank so one
    [69,512] VectorE copy evacuates the whole chunk.
The host sums the partial column stats over blocks/cores and finishes the
O(B + C) scalar math (Pearson means, label gather, log) in float64.
"""
import sys
import types
import numpy as np

sys.path.insert(0, "/opt/trn_rl_repo")

B, C = 4096, 32000
N_CORES = 8
R = B // N_CORES          # 512 rows per core
P = 128                   # partitions
NBLK = R // P             # 4 row blocks per core
CHUNK = 1536
CHUNKS = [(i * CHUNK, CHUNK) for i in range(20)] + [(20 * CHUNK, C - 20 * CHUNK)]
NCH = len(CHUNKS)
EPS = 1e-8

_built = None


def _install_ntff_shim():
    # antenv.axon_hooks is absent in this image; register the ctypes NTFF
    # hook so run_bass_kernel_spmd(trace=True) can profile under axon.
    try:
        import antenv
        import trn_agent_boot.trn_boot as tb
        if "antenv.axon_hooks" in sys.modules:
            return
        hook = tb._ntff_profile_via_ctypes("/opt/axon/libaxon_pjrt.so")
        mod = types.ModuleType("antenv.axon_hooks")
        mod.get_axon_ntff_profile_hook = lambda: hook
        mod.set_axon_ntff_profile_hook = lambda h: None
        antenv.axon_hooks = mod
        sys.modules["antenv.axon_hooks"] = mod
    except Exception:
        pass


def _sub_slices(cw):
    subs = []
    o = 0
    while o < cw:
        n = min(512, cw - o)
        subs.append((o, n))
        o += n
    return subs


def _build():
    from contextlib import ExitStack
    import concourse.bacc as bacc
    import concourse.tile as tile
    from concourse import mybir

    f32 = mybir.dt.float32
    bf16 = mybir.dt.bfloat16
    Exp = mybir.ActivationFunctionType.Exp
    Square = mybir.ActivationFunctionType.Square
    ADD = mybir.AluOpType.add
    AXF = mybir.AxisListType.X

    nc = bacc.Bacc("TRN2", target_bir_lowering=False, debug=False)
    zs_d = nc.dram_tensor("z_s", [R, C], f32, kind="ExternalInput")
    zt_d = nc.dram_tensor("z_t", [R, C], f32, kind="ExternalInput")
    # [block, chunk, psum partition, 512]: rows 32s..32s+4 hold stats 0..4 of
    # sub-matmul s; everything else is don't-care filler the host skips.
    col_d = nc.dram_tensor("colstats", [NBLK, NCH, 69, 512], f32,
                           kind="ExternalOutput")
    row_d = nc.dram_tensor("rowstats", [R, 8], f32, kind="ExternalOutput")

    GRP = 2  # chunks per PE burst group (product tiles buffered GRP+1 deep)

    with tile.TileContext(nc) as tc, ExitStack() as ctx:
        zin = ctx.enter_context(tc.tile_pool(name="zin", bufs=3))
        esp = ctx.enter_context(tc.tile_pool(name="esp", bufs=NCH))
        etp = ctx.enter_context(tc.tile_pool(name="etp", bufs=NCH))
        prod = ctx.enter_context(tc.tile_pool(name="prod", bufs=3 * (GRP + 1)))
        halfp = ctx.enter_context(tc.tile_pool(name="halfp", bufs=4))
        statp = ctx.enter_context(tc.tile_pool(name="stat", bufs=4))
        small = ctx.enter_context(tc.tile_pool(name="small", bufs=2))
        psump = ctx.enter_context(tc.tile_pool(name="psum", bufs=6, space="PSUM"))

        for b in range(NBLK):
            r0 = b * P
            zsp = small.tile([P, NCH], f32, tag="zsp")
            ztp = small.tile([P, NCH], f32, tag="ztp")
            u11p = small.tile([P, NCH], f32, tag="u11p")
            u22p = small.tile([P, NCH], f32, tag="u22p")
            u12p = small.tile([P, NCH], f32, tag="u12p")

            es_tiles = []
            et_tiles = []
            prod_tiles = {}
            for ci, (c0, cw) in enumerate(CHUNKS):
                zs = zin.tile([P, cw], f32, tag="zin")
                nc.sync.dma_start(zs[:], zs_d[r0:r0 + P, c0:c0 + cw])
                es = esp.tile([P, cw], bf16, tag="es")
                nc.scalar.activation(es[:], zs[:], Exp, accum_out=zsp[:, ci:ci + 1])
                zt = zin.tile([P, cw], f32, tag="zin")
                nc.sync.dma_start(zt[:], zt_d[r0:r0 + P, c0:c0 + cw])
                et = etp.tile([P, cw], bf16, tag="et")
                nc.scalar.activation(et[:], zt[:], Exp, accum_out=ztp[:, ci:ci + 1])
                es_tiles.append(es)
                et_tiles.append(et)

            rs = small.tile([P, 8], f32, tag="rs")
            nc.vector.tensor_reduce(rs[:, 0:1], zsp[:, 0:NCH], axis=AXF, op=ADD)
            nc.vector.tensor_reduce(rs[:, 1:2], ztp[:, 0:NCH], axis=AXF, op=ADD)
            w1 = small.tile([P, 1], f32, tag="w1")
            nc.vector.reciprocal(w1[:], rs[:, 0:1])
            w2 = small.tile([P, 1], f32, tag="w2")
            nc.vector.reciprocal(w2[:], rs[:, 1:2])
            # Stat k's weights live in column k of an otherwise-zero [P, 5]
            # stationary tile, so 5 accumulating matmuls (one per stat, each
            # with its own rhs) build a [5, n] PSUM block at base partition
            # 0/32/64 (one per sub-matmul of the chunk).
            W_tiles = []
            for k in range(5):
                Wk = small.tile([P, 5], bf16, tag=f"W{k}")
                nc.vector.memset(Wk[:], 0.0)
                W_tiles.append(Wk)
            nc.vector.tensor_copy(W_tiles[0][:, 0:1], w1[:])
            nc.vector.tensor_copy(W_tiles[1][:, 1:2], w2[:])
            nc.vector.tensor_mul(W_tiles[2][:, 2:3], w1[:], w1[:])
            nc.vector.tensor_mul(W_tiles[3][:, 3:4], w2[:], w2[:])
            nc.vector.tensor_mul(W_tiles[4][:, 4:5], w1[:], w2[:])

            def emit_products(ci):
                c0, cw = CHUNKS[ci]
                es, et = es_tiles[ci], et_tiles[ci]
                p11 = prod.tile([P, cw], bf16, tag="prod")
                nc.scalar.activation(p11[:], es[:], Square,
                                     accum_out=u11p[:, ci:ci + 1])
                p22 = prod.tile([P, cw], bf16, tag="prod")
                nc.vector.tensor_mul(p22[:], et[:], et[:])
                p12 = prod.tile([P, cw], bf16, tag="prod")
                nc.vector.tensor_mul(p12[:], es[:], et[:])
                h = cw // 2
                h22 = halfp.tile([P, h], bf16, tag="half")
                nc.vector.tensor_add(h22[:], p22[:, 0:h], p22[:, h:cw])
                nc.vector.tensor_reduce(u22p[:, ci:ci + 1], h22[:], axis=AXF, op=ADD)
                h12 = halfp.tile([P, h], bf16, tag="half")
                nc.vector.tensor_add(h12[:], p12[:, 0:h], p12[:, h:cw])
                nc.vector.tensor_reduce(u12p[:, ci:ci + 1], h12[:], axis=AXF, op=ADD)
                prod_tiles[ci] = (p11, p22, p12)

            def emit_matmuls(ci):
                c0, cw = CHUNKS[ci]
                es, et = es_tiles[ci], et_tiles[ci]
                p11, p22, p12 = prod_tiles.pop(ci)
                rhs_list = [es, et, p11, p22, p12]
                ps = psump.tile([69, 512], f32, tag="ps")
                for s, (o, n) in enumerate(_sub_slices(cw)):
                    for k in range(5):
                        nc.tensor.matmul(ps[32 * s:32 * s + 5, 0:n],
                                         W_tiles[k][:, 0:5],
                                         rhs_list[k][:, o:o + n],
                                         start=(k == 0), stop=(k == 4))
                st = statp.tile([69, 512], f32, tag="st")
                if ci % 2 == 0:
                    nc.vector.tensor_copy(st[:], ps[:])
                else:
                    nc.scalar.copy(st[:], ps[:])
                nc.sync.dma_start(col_d[b, ci], st[:])

            for g0 in range(0, NCH, GRP):
                group = range(g0, min(g0 + GRP, NCH))
                for ci in group:
                    emit_products(ci)
                for ci in group:
                    emit_matmuls(ci)

            nc.vector.tensor_reduce(rs[:, 2:3], u11p[:, 0:NCH], axis=AXF, op=ADD)
            nc.vector.tensor_reduce(rs[:, 3:4], u22p[:, 0:NCH], axis=AXF, op=ADD)
            nc.vector.tensor_reduce(rs[:, 4:5], u12p[:, 0:NCH], axis=AXF, op=ADD)
            nc.sync.dma_start(row_d[r0:r0 + P, 0:5], rs[:, 0:5])

    nc.compile()
    return nc


def _get_built():
    global _built
    if _built is None:
        _install_ntff_shim()
        _built = _build()
    return _built


def _unpack_col(colstats):
    """colstats [NBLK, NCH, 69, 512] (f32, already summed over cores ok) ->
    [5, C] float64 column stats."""
    acc = colstats.astype(np.float64).sum(axis=0)   # [NCH, 69, 512]
    col = np.zeros((5, C), np.float64)
    for ci, (c0, cw) in enumerate(CHUNKS):
        for s, (o, n) in enumerate(_sub_slices(cw)):
            col[:, c0 + o:c0 + o + n] += acc[ci, 32 * s:32 * s + 5, 0:n]
    return col


def run_sharded(z_s, z_t, trace=False, tmpdir=None):
    """Run the device program; returns (colstats_sum [5, C] f64,
    rowstats [B, 5] f64, BassKernelResults)."""
    from concourse.bass_utils import run_bass_kernel_spmd

    nc = _get_built()
    z_s = np.ascontiguousarray(np.asarray(z_s, dtype=np.float32))
    z_t = np.ascontiguousarray(np.asarray(z_t, dtype=np.float32))
    in_maps = [
        {"z_s": z_s[i * R:(i + 1) * R], "z_t": z_t[i * R:(i + 1) * R]}
        for i in range(N_CORES)
    ]
    res = run_bass_kernel_spmd(nc, in_maps, core_ids=list(range(N_CORES)),
                               trace=trace, tmpdir=tmpdir)
    col = np.zeros((5, C), np.float64)
    rows = []
    for i in range(N_CORES):
        col += _unpack_col(res.results[i]["colstats"])
        rows.append(res.results[i]["rowstats"][:, :5].astype(np.float64))
    return col, np.concatenate(rows, axis=0), res


def kernel(z_s, z_t, labels):
    col, rowstats, _ = run_sharded(z_s, z_t)
    return _finish(np.asarray(z_s), np.asarray(labels), col, rowstats)


def _finish(z_s, labels, col, rowstats):
    Zs, Zt, U11, U22, U12 = rowstats.T
    invC = 1.0 / C
    # inter: Pearson over classes per row (softmax rows have mean 1/C)
    num = U12 / (Zs * Zt) - invC
    vs = U11 / (Zs * Zs) - invC
    vt = U22 / (Zt * Zt) - invC
    corr = num / (np.sqrt(vs) * np.sqrt(vt) + EPS)
    inter = 1.0 - corr.mean()
    # intra: Pearson over samples per column
    S1, S2, S11, S22, S12 = col
    numc = S12 - S1 * S2 / B
    vsc = S11 - S1 * S1 / B
    vtc = S22 - S2 * S2 / B
    corrc = numc / (np.sqrt(vsc) * np.sqrt(vtc) + EPS)
    intra = 1.0 - corrc.mean()
    # hard CE: mean(logsumexp(z_s) - z_s[label])
    lab = np.asarray(labels).astype(np.int64).ravel()
    zl = z_s[np.arange(B), lab].astype(np.float64)
    hard = (np.log(Zs) - zl).mean()
    return np.float32(hard + inter + intra)



# revision 7
# speedup vs baseline: 1.1631x; 1.1631x over previous
"""DIST loss (hard CE + inter/intra Pearson distillation) on 8 Trainium2 cores.

Strategy: data-parallel over the batch dim (4096 rows -> 512 rows/core), with
z_s/z_t cast to bf16 on the host (halves HBM traffic; exp/product sums tolerate
the 2^-8 mantissa easily at the 2e-2 gate).

Per 128-row block each core streams its [128, 32000] bf16 shards once:
  - ScalarE: exp on 8000-wide tiles (accum_out -> per-row Zs/Zt) plus 5 of the
    8 per-block es^2 Square tiles (accum_out -> U11 partials).
  - VectorE: remaining es^2 tiles, et^2, es*et products (bf16 2x mode), the
    2000->1000 halve-add tops of the U22/U12/U11 row-sum chains, reciprocals
    and the 5 weight columns.
  - GpSimd: 1000->500 halve + final 500-col reduces of the row-sum chains, and
    PSUM->SBUF bf16 evacuation of the column-stat matmul results.
  - TensorE: per 2000-col chunk, 4 sub-matmuls (512/512/512/464 wide) x 5
    accumulating stats with per-stat weight columns (1/Zs, 1/Zt, 1/Zs^2,
    1/Zt^2, 1/(Zs*Zt)) as the stationary operand, landing at PSUM partition
    bases 0/32/64/96 of one [128, 512] bank.
The host sums the bf16 column-stat partials over blocks/cores and finishes the
O(B + C) scalar math (Pearson means, label gather, log) in float64; the hard-CE
label gather uses the original f32 z_s.
"""
import sys
import types
import numpy as np

sys.path.insert(0, "/opt/trn_rl_repo")

B, C = 4096, 32000
N_CORES = 8
R = B // N_CORES          # 512 rows per core
P = 128                   # partitions
NBLK = R // P             # 4 row blocks per core
TW = 8000                 # exp tile width
NT = C // TW              # 4 exp tiles per block per tensor
PW = 4000                 # product tile width
NP = C // PW              # 8 product tiles per block
MW = 2000                 # psum chunk width
NM = C // MW              # 16 psum chunks per block
SUBS = [(0, 512), (512, 512), (1024, 512), (1536, 464)]
K_SQ = 5                  # product tiles whose es^2 runs on ScalarE (accum U11)
EPS = 1e-8

_built = None


def _install_ntff_shim():
    # antenv.axon_hooks is absent in this image; register the ctypes NTFF
    # hook so run_bass_kernel_spmd(trace=True) can profile under axon.
    try:
        import antenv
        import trn_agent_boot.trn_boot as tb
        if "antenv.axon_hooks" in sys.modules:
            return
        hook = tb._ntff_profile_via_ctypes("/opt/axon/libaxon_pjrt.so")
        mod = types.ModuleType("antenv.axon_hooks")
        mod.get_axon_ntff_profile_hook = lambda: hook
        mod.set_axon_ntff_profile_hook = lambda h: None
        antenv.axon_hooks = mod
        sys.modules["antenv.axon_hooks"] = mod
    except Exception:
        pass


def _build():
    from contextlib import ExitStack
    import concourse.bacc as bacc
    import concourse.tile as tile
    from concourse import mybir

    f32 = mybir.dt.float32
    bf16 = mybir.dt.bfloat16
    Exp = mybir.ActivationFunctionType.Exp
    Square = mybir.ActivationFunctionType.Square
    ADD = mybir.AluOpType.add
    AXF = mybir.AxisListType.X

    nc = bacc.Bacc("TRN2", target_bir_lowering=False, debug=False)
    zs_d = nc.dram_tensor("z_s", [R, C], bf16, kind="ExternalInput")
    zt_d = nc.dram_tensor("z_t", [R, C], bf16, kind="ExternalInput")
    # per (block, 2000-col chunk): the full [128, 512] psum bank in bf16; host
    # picks rows 32*s + k (sub s, stat k) and ignores the rest.
    col_d = nc.dram_tensor("colstats", [NBLK, NM, P, 512], bf16,
                           kind="ExternalOutput")
    row_d = nc.dram_tensor("rowstats", [R, 8], f32, kind="ExternalOutput")

    with tile.TileContext(nc) as tc, ExitStack() as ctx:
        zin = ctx.enter_context(tc.tile_pool(name="zin", bufs=2))
        esp = ctx.enter_context(tc.tile_pool(name="esp", bufs=NT))
        etp = ctx.enter_context(tc.tile_pool(name="etp", bufs=NT))
        prod = ctx.enter_context(tc.tile_pool(name="prod", bufs=4))
        h1p = ctx.enter_context(tc.tile_pool(name="h1p", bufs=2))
        h2p = ctx.enter_context(tc.tile_pool(name="h2p", bufs=2))
        h3p = ctx.enter_context(tc.tile_pool(name="h3p", bufs=2))
        h4p = ctx.enter_context(tc.tile_pool(name="h4p", bufs=2))
        stp = ctx.enter_context(tc.tile_pool(name="stp", bufs=3))
        wtp = ctx.enter_context(tc.tile_pool(name="wtp", bufs=10))
        small = ctx.enter_context(tc.tile_pool(name="small", bufs=2))
        psump = ctx.enter_context(tc.tile_pool(name="psum", bufs=6, space="PSUM"))

        # sync-queue DMA dispatch order is program order; stagger the output
        # DMAs one block behind the input DMAs so next-block input dispatch
        # never waits on this block's evacuations.
        out_dma_q = []

        for b in range(NBLK):
            r0 = b * P
            zsp = small.tile([P, NT], f32, tag="zsp")
            ztp = small.tile([P, NT], f32, tag="ztp")
            u11p = small.tile([P, NP], f32, tag="u11p")
            u22p = small.tile([P, NP], f32, tag="u22p")
            u12p = small.tile([P, NP], f32, tag="u12p")

            es_tiles = []
            et_tiles = []
            for t in range(NT):
                c0 = t * TW
                zs = zin.tile([P, TW], bf16, tag="zin")
                nc.sync.dma_start(zs[:], zs_d[r0:r0 + P, c0:c0 + TW])
                es = esp.tile([P, TW], bf16, tag="es")
                nc.scalar.activation(es[:], zs[:], Exp, accum_out=zsp[:, t:t + 1])
                zt = zin.tile([P, TW], bf16, tag="zin")
                nc.sync.dma_start(zt[:], zt_d[r0:r0 + P, c0:c0 + TW])
                et = etp.tile([P, TW], bf16, tag="et")
                nc.scalar.activation(et[:], zt[:], Exp, accum_out=ztp[:, t:t + 1])
                es_tiles.append(es)
                et_tiles.append(et)

            # flush the previous block's output DMAs now (after this block's
            # input dispatch) to keep the sync queue from stalling inputs.
            for fn in out_dma_q:
                fn()
            out_dma_q = []

            rs = small.tile([P, 8], f32, tag="rs")
            nc.vector.tensor_reduce(rs[:, 0:1], zsp[:, 0:NT], axis=AXF, op=ADD)
            nc.vector.tensor_reduce(rs[:, 1:2], ztp[:, 0:NT], axis=AXF, op=ADD)
            w1 = small.tile([P, 1], f32, tag="w1")
            nc.vector.reciprocal(w1[:], rs[:, 0:1])
            w2 = small.tile([P, 1], f32, tag="w2")
            nc.vector.reciprocal(w2[:], rs[:, 1:2])
            W_tiles = []
            for k in range(5):
                Wk = wtp.tile([P, 8], bf16, tag=f"W{k}")
                nc.vector.memset(Wk[:], 0.0)
                W_tiles.append(Wk)
            nc.vector.tensor_copy(W_tiles[0][:, 0:1], w1[:])
            nc.vector.tensor_copy(W_tiles[1][:, 1:2], w2[:])
            nc.vector.tensor_mul(W_tiles[2][:, 2:3], w1[:], w1[:])
            nc.vector.tensor_mul(W_tiles[3][:, 3:4], w2[:], w2[:])
            nc.vector.tensor_mul(W_tiles[4][:, 4:5], w1[:], w2[:])

            def chain(p, col, upcol, top_engine=None):
                # row-sum of a [P, PW] bf16 product tile into upcol[:, col]
                h1 = h1p.tile([P, PW // 2], bf16, tag="h1")
                nc.vector.tensor_add(h1[:], p[:, 0:PW // 2], p[:, PW // 2:PW])
                h2 = h2p.tile([P, PW // 4], bf16, tag="h2")
                nc.vector.tensor_add(h2[:], h1[:, 0:PW // 4], h1[:, PW // 4:PW // 2])
                h3 = h3p.tile([P, PW // 8], bf16, tag="h3")
                nc.gpsimd.tensor_add(h3[:], h2[:, 0:PW // 8], h2[:, PW // 8:PW // 4])
                h4 = h4p.tile([P, PW // 16], bf16, tag="h4")
                nc.gpsimd.tensor_add(h4[:], h3[:, 0:PW // 16], h3[:, PW // 16:PW // 8])
                nc.vector.tensor_reduce(upcol[:, col:col + 1], h4[:], axis=AXF,
                                        op=ADD)

            for j in range(NP):
                t, half = j // 2, j % 2
                es_sl = es_tiles[t][:, half * PW:(half + 1) * PW]
                et_sl = et_tiles[t][:, half * PW:(half + 1) * PW]
                p11 = prod.tile([P, PW], bf16, tag="prod")
                if j < K_SQ:
                    nc.scalar.activation(p11[:], es_sl, Square,
                                         accum_out=u11p[:, j:j + 1])
                else:
                    nc.vector.tensor_mul(p11[:], es_sl, es_sl)
                p22 = prod.tile([P, PW], bf16, tag="prod")
                nc.vector.tensor_mul(p22[:], et_sl, et_sl)
                p12 = prod.tile([P, PW], bf16, tag="prod")
                nc.vector.tensor_mul(p12[:], es_sl, et_sl)
                if j >= K_SQ:
                    chain(p11, j, u11p)
                chain(p22, j, u22p)
                chain(p12, j, u12p)

                for hh in range(2):
                    m = 2 * j + hh
                    off = hh * MW
                    rhs = [es_sl, et_sl, p11, p22, p12]
                    ps = psump.tile([P, 512], f32, tag="ps")
                    for s, (o, w) in enumerate(SUBS):
                        for k in range(5):
                            nc.tensor.matmul(ps[32 * s:32 * s + 5, 0:w],
                                             W_tiles[k][:, 0:5],
                                             rhs[k][:, off + o:off + o + w],
                                             start=(k == 0), stop=(k == 4),
                                             tile_position=(0, 32 * s))
                    st = stp.tile([P, 512], bf16, tag="st")
                    if m % 2 == 0:
                        nc.scalar.copy(st[:], ps[:])
                    else:
                        nc.vector.tensor_copy(st[:], ps[:])

                    def emit_out(bb=b, mm=m, stt=st):
                        nc.sync.dma_start(col_d[bb, mm], stt[:])
                    out_dma_q.append(emit_out)

            nc.vector.tensor_reduce(rs[:, 2:3], u11p[:, 0:NP], axis=AXF, op=ADD)
            nc.vector.tensor_reduce(rs[:, 3:4], u22p[:, 0:NP], axis=AXF, op=ADD)
            nc.vector.tensor_reduce(rs[:, 4:5], u12p[:, 0:NP], axis=AXF, op=ADD)

            def emit_rs(bb=b, rss=rs):
                nc.sync.dma_start(row_d[bb * P:bb * P + P, 0:8], rss[:])
            out_dma_q.append(emit_rs)

        for fn in out_dma_q:
            fn()

    nc.compile()
    return nc


def _get_built():
    global _built
    if _built is None:
        _install_ntff_shim()
        _built = _build()
    return _built


def _unpack_col(colstats):
    """colstats [NBLK, NM, 128, 512] bf16 -> [5, C] float64 column stats."""
    acc = np.asarray(colstats).astype(np.float64).sum(axis=0)  # [NM, 128, 512]
    col = np.zeros((5, C), np.float64)
    for m in range(NM):
        c0 = m * MW
        for s, (o, w) in enumerate(SUBS):
            for k in range(5):
                col[k, c0 + o:c0 + o + w] += acc[m, 32 * s + k, 0:w]
    return col


def run_sharded(z_s, z_t, trace=False, tmpdir=None):
    """Run the device program; returns (colstats_sum [5, C] f64,
    rowstats [B, 5] f64, BassKernelResults)."""
    import ml_dtypes
    from concourse.bass_utils import run_bass_kernel_spmd

    nc = _get_built()
    bf16 = ml_dtypes.bfloat16
    z_s = np.ascontiguousarray(np.asarray(z_s, dtype=np.float32).astype(bf16))
    z_t = np.ascontiguousarray(np.asarray(z_t, dtype=np.float32).astype(bf16))
    in_maps = [
        {"z_s": z_s[i * R:(i + 1) * R], "z_t": z_t[i * R:(i + 1) * R]}
        for i in range(N_CORES)
    ]
    res = run_bass_kernel_spmd(nc, in_maps, core_ids=list(range(N_CORES)),
                               trace=trace, tmpdir=tmpdir)
    col = np.zeros((5, C), np.float64)
    rows = []
    for i in range(N_CORES):
        col += _unpack_col(res.results[i]["colstats"])
        rows.append(res.results[i]["rowstats"][:, :5].astype(np.float64))
    return col, np.concatenate(rows, axis=0), res


def kernel(z_s, z_t, labels):
    col, rowstats, _ = run_sharded(z_s, z_t)
    return _finish(np.asarray(z_s), np.asarray(labels), col, rowstats)


def _finish(z_s, labels, col, rowstats):
    Zs, Zt, U11, U22, U12 = rowstats.T
    invC = 1.0 / C
    # inter: Pearson over classes per row (softmax rows have mean 1/C)
    num = U12 / (Zs * Zt) - invC
    vs = U11 / (Zs * Zs) - invC
    vt = U22 / (Zt * Zt) - invC
    corr = num / (np.sqrt(vs) * np.sqrt(vt) + EPS)
    inter = 1.0 - corr.mean()
    # intra: Pearson over samples per column
    S1, S2, S11, S22, S12 = col
    numc = S12 - S1 * S2 / B
    vsc = S11 - S1 * S1 / B
    vtc = S22 - S2 * S2 / B
    corrc = numc / (np.sqrt(vsc) * np.sqrt(vtc) + EPS)
    intra = 1.0 - corrc.mean()
    # hard CE: mean(logsumexp(z_s) - z_s[label])
    lab = np.asarray(labels).astype(np.int64).ravel()
    zl = z_s[np.arange(B), lab].astype(np.float64)
    hard = (np.log(Zs) - zl).mean()
    return np.float32(hard + inter + intra)
